# revision 1
# baseline (speedup 1.0000x reference)
"""Cached self-attention Trainium2 kernel (v3).

Sharding: 8 cores = 2 batches x 4 head-groups. Core c: batch b=c//4, group
g=c%4 owns heads 4g..4g+3 (columns 512g:512g+512 of the q/k/v projections).
Each core projects q/k/v for its heads over the full sequence, runs attention
for its 4 heads, the 4 cores of a batch AllGather the (normalized, transposed)
per-head attention outputs, and each core computes the output projection onto
its 512-column slice of wo (full sequence), so outputs tile the model dim.

All matmuls fp16 x fp16 -> fp32 PSUM. Softmax: exp on ScalarE with the
1/sqrt(128) scale folded into the q evacuation; Z via fp16 DVE running adds +
ones-matmul cross-partition sum + fast reciprocal; normalization applied
during PSUM evacuation of the attention output.
"""
import numpy as np
from contextlib import ExitStack

import concourse.bass as bass
import concourse.tile as tile
from concourse import bacc, mybir
from concourse.bass_utils import run_bass_kernel_spmd

B, S, PC, D, H = 2, 2048, 2048, 2048, 16
HD = D // H            # 128 head dim
GH = H // 4            # 4 heads per core
DG = GH * HD           # 512 head-dims per core
NB = 512               # block size
NKC = (PC + S) // HD   # 32 key chunks of 128
NDC = D // HD          # 16 contraction chunks
F16 = mybir.dt.float16
F32 = mybir.dt.float32
AF = mybir.ActivationFunctionType
ALU = mybir.AluOpType
INV_SQRT_HD = float(1.0 / np.sqrt(HD))

GROUPS = [[0, 1, 2, 3], [4, 5, 6, 7]]


def build():
    nc = bacc.Bacc("TRN2", target_bir_lowering=False, debug=False, num_devices=8)

    def inp(name, shape):
        return nc.dram_tensor(name, shape, F16, kind="ExternalInput").ap()

    xT = inp("xT", [D, S])          # x[b].T
    wq = inp("wq", [D, DG])         # wq[:, 512g:512g+512]
    bq = inp("bq", [DG])            # bq slice / sqrt(HD)
    wk = inp("wk", [D, DG])
    bk = inp("bk", [DG])
    wv = inp("wv", [D, DG])
    bv = inp("bv", [DG])
    ckT = inp("ckT", [DG, PC])      # cache_k[b,:,slice].T
    cv = inp("cv", [PC, DG])        # cache_v[b,:,slice]
    wo = inp("wo", [D, DG])         # wo rows permuted to gather order, cols sliced
    bo = inp("bo", [DG])
    y = nc.dram_tensor("y", [S, DG], F32, kind="ExternalOutput").ap()

    with tile.TileContext(nc) as tc, ExitStack() as ctx:
        res = ctx.enter_context(tc.tile_pool(name="res", bufs=1))
        dram = ctx.enter_context(tc.tile_pool(name="dram", bufs=1, space="DRAM"))

        # tiny whole-kernel residents
        bq_t = res.tile([HD, GH], F16, tag="bq")
        bk_t = res.tile([HD, GH], F16, tag="bk")
        bv_t = res.tile([1, DG], F16, tag="bv")
        bo_t = res.tile([1, DG], F16, tag="bo")
        ones_k = res.tile([HD, 1], F16, tag="ones_k")      # [128,1] ones
        ones_r16 = res.tile([1, HD], F16, tag="ones_r16")  # [1,128] ones
        ones_r32 = res.tile([1, HD], F32, tag="ones_r32")
        nc.sync.dma_start(bq_t[:], bq.rearrange("(m p) -> p m", p=HD))
        nc.sync.dma_start(bk_t[:], bk.rearrange("(m p) -> p m", p=HD))
        nc.sync.dma_start(bv_t[:], bv[None, :])
        nc.sync.dma_start(bo_t[:], bo[None, :])
        nc.vector.memset(ones_k[:], 1.0)
        nc.vector.memset(ones_r16[:], 1.0)
        nc.vector.memset(ones_r32[:], 1.0)

        # collective bounce buffers
        bounce_in = []
        bounce_out = []
        for j in range(GH):
            bounce_in.append(dram.tile([HD, GH, NB], F16, tag=f"bi{j}",
                                       name=f"bi{j}"))
            bounce_out.append(dram.tile([4, HD, GH, NB], F16, tag=f"bg{j}",
                                        name=f"bg{j}"))

        with ExitStack() as c12:
            # phase 1+2 residents
            ph = c12.enter_context(tc.tile_pool(name="ph", bufs=1))
            qT = ph.tile([HD, GH, S], F16, tag="qT")        # [128, 4, 2048]
            kTn = ph.tile([HD, GH, S], F16, tag="kTn")
            ckT_t = ph.tile([HD, GH, PC], F16, tag="ckT")
            cv_t = ph.tile([HD, PC // HD, DG], F16, tag="cv")   # [128, 16, 512]
            vn_t = ph.tile([HD, S // HD, DG], F16, tag="vn")
            nc.sync.dma_start(ckT_t[:], ckT.rearrange("(m p) s -> p m s", p=HD))
            nc.sync.dma_start(cv_t[:], cv.rearrange("(ss p) d -> p ss d", p=HD))

            # ---- phase 1: projections ----
            with tc.tile_pool(name="px", bufs=1) as px, \
                 tc.tile_pool(name="pw", bufs=2) as pw, \
                 tc.tile_pool(name="ps1", bufs=1, space="PSUM") as ps1:
                xres = px.tile([HD, NDC, S], F16, tag="xres")   # 8.4 MB
                xr = xT.rearrange("(kc p) s -> p kc s", p=HD)
                for kq in range(4):
                    nc.sync.dma_start(xres[:, 4 * kq:4 * (kq + 1), :],
                                      xr[:, 4 * kq:4 * (kq + 1), :])
                wvt = px.tile([HD, NDC, DG], F16, tag="wvt")    # 2.1 MB
                nc.sync.dma_start(wvt[:],
                                  wv.rearrange("(kc p) n -> p kc n", p=HD))

                # q pass then k pass: weights stay loaded across the 4 s-blocks
                for wsrc, dst, bias_t, scale in (
                        (wq, qT, bq_t, INV_SQRT_HD), (wk, kTn, bk_t, 1.0)):
                    for m in range(GH):
                        wt = pw.tile([HD, NDC, HD], F16, tag="wqk", name="wt")
                        nc.sync.dma_start(
                            wt[:], wsrc[:, HD * m:HD * (m + 1)].rearrange(
                                "(kc p) n -> p kc n", p=HD))
                        psq = [ps1.tile([HD, NB], F32,
                                        tag=f"pp{4 * (m % 2) + sb}",
                                        name=f"psq{sb}") for sb in range(4)]
                        for kc in range(NDC):
                            for sb in range(4):
                                nc.tensor.matmul(
                                    psq[sb][:], wt[:, kc, :],
                                    xres[:, kc, NB * sb:NB * (sb + 1)],
                                    start=(kc == 0), stop=(kc == NDC - 1))
                        for sb in range(4):
                            nc.scalar.activation(
                                dst[:, m, NB * sb:NB * (sb + 1)], psq[sb][:],
                                AF.Identity, bias=bias_t[:, m:m + 1], scale=scale)

                # v pass (natural layout)
                for ss in range(S // HD):
                    psv = ps1.tile([HD, DG], F32, tag=f"pp{ss % 8}", name="psv")
                    for kc in range(NDC):
                        nc.tensor.matmul(psv[:],
                                         xres[:, kc, HD * ss:HD * (ss + 1)],
                                         wvt[:, kc, :],
                                         start=(kc == 0), stop=False)
                    nc.tensor.matmul(psv[:], ones_r16[:], bv_t[:],
                                     start=False, stop=True)
                    nc.any.tensor_copy(vn_t[:, ss, :], psv[:])

            # ---- phase 2: attention per head + AllGather ----
            with tc.tile_pool(name="p2", bufs=6) as p2, \
                 tc.tile_pool(name="zp", bufs=2) as zp, \
                 tc.tile_pool(name="ap", bufs=2) as apool, \
                 tc.tile_pool(name="ps2", bufs=1, space="PSUM") as ps2:
                for j in range(GH):
                    head_scope = nc.named_scope(f"head{j}")
                    head_scope.__enter__()
                    ahead = apool.tile([HD, GH, NB], F16, tag="ah")
                    for sb in range(4):
                        PA = ps2.tile([HD, NB], F32, tag="PA", name="PA")
                        zacc = zp.tile([HD, NB], F16, tag="z")
                        qTs = qT[:, j, NB * sb:NB * (sb + 1)]
                        for c2 in range(NKC // 2):
                            pss = ps2.tile([HD, 2, NB], F32,
                                           tag=f"psS{c2 % 3}", name="pss")
                            e2 = p2.tile([HD, 2, NB], F16, tag="e")
                            for i in range(2):
                                c = 2 * c2 + i
                                if c < PC // HD:
                                    kt = ckT_t[:, j, HD * c:HD * (c + 1)]
                                else:
                                    cc = c - PC // HD
                                    kt = kTn[:, j, HD * cc:HD * (cc + 1)]
                                nc.tensor.matmul(pss[:, i, :], kt, qTs,
                                                 start=True, stop=True)
                            nc.scalar.activation(e2[:], pss[:], AF.Exp)
                            for i in range(2):
                                c = 2 * c2 + i
                                if c < PC // HD:
                                    vt = cv_t[:, c, HD * j:HD * (j + 1)]
                                else:
                                    vt = vn_t[:, c - PC // HD,
                                              HD * j:HD * (j + 1)]
                                nc.tensor.matmul(PA[:], vt, e2[:, i, :],
                                                 start=(c == 0),
                                                 stop=(c == NKC - 1),
                                                 skip_group_check=True)
                            if c2 == 0:
                                nc.vector.tensor_tensor(zacc[:], e2[:, 0, :],
                                                        e2[:, 1, :], ALU.add)
                            else:
                                nc.vector.tensor_tensor(zacc[:], zacc[:],
                                                        e2[:, 0, :], ALU.add)
                                nc.vector.tensor_tensor(zacc[:], zacc[:],
                                                        e2[:, 1, :], ALU.add)
                        psz = ps2.tile([1, NB], F32, tag="psS0", name="psz")
                        nc.tensor.matmul(psz[:], ones_k[:], zacc[:],
                                         start=True, stop=True)
                        zinv = zp.tile([1, NB], F32, tag="zi")
                        nc.vector.reciprocal_approx_fast(zinv[:], psz[:])
                        psb = ps2.tile([HD, NB], F32, tag="psS1", name="psb")
                        nc.tensor.matmul(psb[:], ones_r32[:], zinv[:],
                                         start=True, stop=True)
                        zb = zp.tile([HD, NB], F32, tag="zb")
                        nc.vector.tensor_copy(zb[:], psb[:])
                        nc.vector.tensor_tensor(ahead[:, sb, :], PA[:], zb[:],
                                                ALU.mult)
                    nc.sync.dma_start(bounce_in[j][:], ahead[:])
                    nc.gpsimd.collective_compute(
                        "AllGather", ALU.bypass, replica_groups=GROUPS,
                        ins=[bounce_in[j].opt()], outs=[bounce_out[j].opt()])
                    head_scope.__exit__(None, None, None)

        # ---- phase 3: output projection (full sequence, 512-col wo slice) ----
        with tc.tile_pool(name="p3", bufs=3) as p3, \
             tc.tile_pool(name="lt3", bufs=1) as ltp, \
             tc.tile_pool(name="wo3", bufs=1) as wop, \
             tc.tile_pool(name="ps3", bufs=1, space="PSUM") as ps3:
            wot = wop.tile([HD, 16, NB], F16, tag="wo")
            nc.sync.dma_start(wot[:], wo.rearrange("(c p) n -> p c n", p=HD))
            # one big load per (j, r): [128, 4, 512] contiguous in the bounce
            lts = []
            for j in range(GH):
                for r in range(4):
                    lt = ltp.tile([HD, GH, NB], F16, tag=f"lt{4 * j + r}",
                                  name=f"lt{4 * j + r}")
                    nc.sync.dma_start(lt[:], bounce_out[j][r])
                    lts.append(lt)
            for m in range(S // HD):
                psO = ps3.tile([HD, NB], F32, tag=f"psO{m % 2}", name="psO")
                for jr in range(16):
                    nc.tensor.matmul(
                        psO[:],
                        lts[jr][:, m // 4, HD * (m % 4):HD * (m % 4 + 1)],
                        wot[:, jr, :],
                        start=(jr == 0), stop=False, skip_group_check=True)
                nc.tensor.matmul(psO[:], ones_r16[:], bo_t[:],
                                 start=False, stop=True, skip_group_check=True)
                ot = p3.tile([HD, NB], F32, tag="ot")
                nc.any.tensor_copy(ot[:], psO[:])
                nc.sync.dma_start(y[HD * m:HD * (m + 1), :], ot[:])

    nc.compile()
    return nc


_BUILT = None


def get_built():
    global _BUILT
    if _BUILT is None:
        _BUILT = build()
    return _BUILT


def make_in_maps(x, cache_k, cache_v, wq, bq, wk, bk, wv, bv, wo, bo):
    x = np.asarray(x)
    cache_k = np.asarray(cache_k)
    cache_v = np.asarray(cache_v)
    wq, bq = np.asarray(wq), np.asarray(bq)
    wk, bk = np.asarray(wk), np.asarray(bk)
    wv, bv = np.asarray(wv), np.asarray(bv)
    wo, bo = np.asarray(wo), np.asarray(bo)

    # permute wo rows to match gather order: lhsT chunk jr=(4j+r) holds head 4r+j
    perm = np.concatenate([
        np.arange(HD * (4 * r + j), HD * (4 * r + j) + HD)
        for j in range(GH) for r in range(4)
    ])
    wo_p = wo[perm, :]

    in_maps = []
    for c in range(8):
        b, g = divmod(c, 4)
        sl = slice(DG * g, DG * (g + 1))
        in_maps.append({
            "xT": np.ascontiguousarray(x[b].T).astype(np.float16),
            "wq": wq[:, sl].astype(np.float16),
            "bq": (bq[sl] * INV_SQRT_HD).astype(np.float16),
            "wk": wk[:, sl].astype(np.float16),
            "bk": bk[sl].astype(np.float16),
            "wv": wv[:, sl].astype(np.float16),
            "bv": bv[sl].astype(np.float16),
            "ckT": np.ascontiguousarray(cache_k[b][:, sl].T).astype(np.float16),
            "cv": cache_v[b][:, sl].astype(np.float16),
            "wo": wo_p[:, sl].astype(np.float16),
            "bo": bo[sl].astype(np.float16),
        })
    return in_maps


def assemble(results):
    out = np.empty((B, S, D), np.float32)
    for c in range(8):
        b, g = divmod(c, 4)
        out[b, :, DG * g:DG * (g + 1)] = results[c]["y"]
    return out


def kernel(**inputs):
    nc = get_built()
    in_maps = make_in_maps(**inputs)
    res = run_bass_kernel_spmd(nc, in_maps, core_ids=list(range(8)))
    return assemble(res.results)



# revision 6
# speedup vs baseline: 1.1759x; 1.1759x over previous
"""Cached self-attention Trainium2 kernel (v5).

Sharding: 8 cores = 2 batches x 4 head-groups. Core c: batch b=c//4, group
g=c%4 owns heads 4g..4g+3 (columns 512g:512g+512 of the q/k/v projections).

v5 restructure vs v3 (737us baseline):
- DMA order: x + weights first, cache K/V after (x gates all projections).
- k-pass is seq-block-major so it streams behind the x DMA.
- q projections are fused into the attention stream: head j+1's 64
  projection matmuls are interleaved 1-per-c8-iteration into head j's
  attention, so ScalarE exp (the phase-2 co-bottleneck) overlaps TensorE
  projection work and the PE never idles at phase boundaries.
- softmax-Z finalization (ones-matmul partition reduce + reciprocal +
  broadcast matmul) for block (j,sb) is emitted ~2 iterations INTO the
  next block's matmul stream, so TensorE no longer stalls on the DVE
  reciprocal chain (this stall caused a HAM re-throttle every 22us in
  the baseline).
- AllGather is per (head, seq-block): 16x 128KB gathers overlap the
  attention window; phase 3 starts immediately after the last attention
  matmul with no collective trough.
- PSUM: pss0(2) pss1(2) PA0(1) PA1(1) psq(1) z(1) = 8 banks exactly.

All matmuls fp16 x fp16 -> fp32 PSUM. exp on ScalarE at FD=1024 with the
1/sqrt(128) scale folded into the q evacuation.
"""
import numpy as np
from contextlib import ExitStack

import concourse.bass as bass
import concourse.tile as tile
from concourse import bacc, mybir
from concourse.bass_utils import run_bass_kernel_spmd

B, S, PC, D, H = 2, 2048, 2048, 2048, 16
HD = D // H            # 128 head dim
GH = H // 4            # 4 heads per core
DG = GH * HD           # 512 head-dims per core
NB = 512               # seq block size
NDC = D // HD          # 16 contraction chunks
NCC = PC // HD         # 16 cache key chunks
NKC = (PC + S) // HD   # 32 total key chunks
F16 = mybir.dt.float16
F32 = mybir.dt.float32
AF = mybir.ActivationFunctionType
ALU = mybir.AluOpType
INV_SQRT_HD = float(1.0 / np.sqrt(HD))

GROUPS = [[0, 1, 2, 3], [4, 5, 6, 7]]


def build():
    nc = bacc.Bacc("TRN2", target_bir_lowering=False, debug=False, num_devices=8)

    def inp(name, shape):
        return nc.dram_tensor(name, shape, F16, kind="ExternalInput").ap()

    xT = inp("xT", [D, S])          # x[b].T
    wq = inp("wq", [D, DG])         # wq[:, 512g:512g+512]
    bq = inp("bq", [DG])            # bq slice / sqrt(HD)
    wk = inp("wk", [D, DG])
    bk = inp("bk", [DG])
    wv = inp("wv", [D, DG])
    bv = inp("bv", [DG])
    ckT = inp("ckT", [DG, PC])      # cache_k[b,:,slice].T
    cv = inp("cv", [PC, DG])        # cache_v[b,:,slice]
    wo = inp("wo", [D, DG])         # wo rows permuted to gather order, cols sliced
    bo = inp("bo", [DG])
    y = nc.dram_tensor("y", [S, DG], F32, kind="ExternalOutput").ap()

    with tile.TileContext(nc) as tc, ExitStack() as ctx:
        res = ctx.enter_context(tc.tile_pool(name="res", bufs=1))
        dram = ctx.enter_context(tc.tile_pool(name="dram", bufs=1, space="DRAM"))

        # tiny whole-kernel residents (issued first: needed by early evacs)
        bq_t = res.tile([HD, GH], F16, tag="bq")
        bk_t = res.tile([HD, GH], F16, tag="bk")
        bv_t = res.tile([1, DG], F16, tag="bv")
        bo_t = res.tile([1, DG], F16, tag="bo")
        ones_k = res.tile([HD, 1], F16, tag="ones_k")      # [128,1] ones
        ones_r16 = res.tile([1, HD], F16, tag="ones_r16")  # [1,128] ones
        ones_r32 = res.tile([1, HD], F32, tag="ones_r32")
        nc.sync.dma_start(bq_t[:], bq.rearrange("(m p) -> p m", p=HD))
        nc.sync.dma_start(bk_t[:], bk.rearrange("(m p) -> p m", p=HD))
        nc.sync.dma_start(bv_t[:], bv[None, :])
        nc.sync.dma_start(bo_t[:], bo[None, :])
        nc.vector.memset(ones_k[:], 1.0)
        nc.vector.memset(ones_r16[:], 1.0)
        nc.vector.memset(ones_r32[:], 1.0)

        # big residents for phases 1+2
        kTn = res.tile([HD, GH, S], F16, tag="kTn")         # 16KB/p
        vn_t = res.tile([HD, S // HD, DG], F16, tag="vn")   # 16KB/p
        cv_t = res.tile([HD, NCC, DG], F16, tag="cv")       # 16KB/p
        ckT_t = res.tile([HD, GH, PC], F16, tag="ckT")      # 16KB/p

        # attention working pools (whole-kernel scope)
        qp = ctx.enter_context(tc.tile_pool(name="qp", bufs=2))
        ep = ctx.enter_context(tc.tile_pool(name="ep", bufs=3))
        zp = ctx.enter_context(tc.tile_pool(name="zp", bufs=2))
        apool = ctx.enter_context(tc.tile_pool(name="apool", bufs=2))

        # collective bounce buffers, per (head, seq-block)
        bounce_in = [[dram.tile([HD, NB], F16, tag=f"bi{j}_{sb}",
                                name=f"bi{j}_{sb}") for sb in range(4)]
                     for j in range(GH)]
        bounce_out = [[dram.tile([4, HD, NB], F16, tag=f"bg{j}_{sb}",
                                 name=f"bg{j}_{sb}") for sb in range(4)]
                      for j in range(GH)]

        qT_tiles = {}
        lts = {}
        pend = [None]          # deferred z-finalize for the previous (j,sb)
        lt_pool = [None]       # set once the phase-3 lt pool is open

        def emit_pending():
            if pend[0] is not None:
                pend[0][0]()
                pend[0][1]()
                pend[0] = None

        def make_z(pB, j, sb, zfold, PA):
            st = {}

            def go1():
                # partition-reduce Z; emitted early in the NEXT block's
                # matmul stream so TensorE never waits on the DVE chain
                psz = pB.tile([1, NB], F32, tag="z", name="psz")
                nc.tensor.matmul(psz[:], ones_k[:], zfold[:],
                                 start=True, stop=True, skip_group_check=True)
                zinv = zp.tile([1, NB], F32, tag="zi")
                nc.vector.reciprocal_approx_fast(zinv[:], psz[:])
                st["zinv"] = zinv

            def go2():
                # broadcast 1/Z across partitions, normalize, gather
                psb = pB.tile([HD, NB], F32, tag="z", name="psb")
                nc.tensor.matmul(psb[:], ones_r32[:], st["zinv"][:],
                                 start=True, stop=True, skip_group_check=True)
                zb = zp.tile([HD, NB], F32, tag="zb", bufs=1)
                nc.vector.tensor_copy(zb[:], psb[:])
                ahead = apool.tile([HD, NB], F16, tag="ah")
                nc.vector.tensor_tensor(ahead[:], PA[:], zb[:], ALU.mult)
                nc.sync.dma_start(bounce_in[j][sb][:], ahead[:])
                nc.gpsimd.collective_compute(
                    "AllGather", ALU.bypass, replica_groups=GROUPS,
                    ins=[bounce_in[j][sb].opt()], outs=[bounce_out[j][sb].opt()])
                if lt_pool[0] is not None:
                    # phase-3 lhsT for this block: load as soon as gathered
                    lt = lt_pool[0].tile([HD, 4, NB], F16, tag=f"lt{j}_{sb}",
                                         name=f"lt{j}_{sb}")
                    nc.sync.dma_start(
                        lt[:], bounce_out[j][sb].rearrange("r p n -> p r n"))
                    lts[(j, sb)] = lt
            return go1, go2

        def make_qproj(jq, pool, wqt, xres):
            qt = qp.tile([HD, S], F16, tag="qT", name=f"qT{jq}")
            qT_tiles[jq] = qt
            cur = {}

            def step(s):
                sbq, kc = divmod(s, NDC)
                if kc == 0:
                    cur["psq"] = pool.tile([HD, NB], F32, tag="psq", name="psq")
                psq = cur["psq"]
                nc.tensor.matmul(psq[:], wqt[:, kc, HD * jq:HD * (jq + 1)],
                                 xres[:, kc, NB * sbq:NB * (sbq + 1)],
                                 start=(kc == 0), stop=(kc == NDC - 1),
                                 skip_group_check=True)
                if kc == NDC - 1:
                    nc.scalar.activation(qt[:, NB * sbq:NB * (sbq + 1)], psq[:],
                                         AF.Identity, bias=bq_t[:, jq:jq + 1],
                                         scale=INV_SQRT_HD)
            return step

        def att_head(pB, j, qnext):
            scope = nc.named_scope(f"h{j}")
            scope.__enter__()
            for sb in range(4):
                PA = pB.tile([HD, NB], F32, tag=f"PA{sb % 2}", name="PA")
                zacc2 = zp.tile([HD, 2, NB], F16, tag="za")
                qTs = qT_tiles[j][:, NB * sb:NB * (sb + 1)]
                for c8 in range(NKC // 2):
                    pss = pB.tile([HD, 2, NB], F32, tag=f"pss{c8 % 2}",
                                  name="pss")
                    for i in range(2):
                        c = 2 * c8 + i
                        if c < NCC:
                            kt = ckT_t[:, j, HD * c:HD * (c + 1)]
                        else:
                            kt = kTn[:, j, HD * (c - NCC):HD * (c - NCC + 1)]
                        nc.tensor.matmul(pss[:, i, :], kt, qTs,
                                         start=True, stop=True)
                    e2 = ep.tile([HD, 2, NB], F16, tag="e2")
                    nc.scalar.activation(e2[:], pss[:], AF.Exp)
                    for i in range(2):
                        c = 2 * c8 + i
                        if c < NCC:
                            vt = cv_t[:, c, HD * j:HD * (j + 1)]
                        else:
                            vt = vn_t[:, c - NCC, HD * j:HD * (j + 1)]
                        nc.tensor.matmul(PA[:], vt, e2[:, i, :],
                                         start=(c == 0), stop=(c == NKC - 1),
                                         skip_group_check=True)
                    if c8 == 0:
                        nc.vector.tensor_copy(zacc2[:], e2[:])
                    else:
                        nc.vector.tensor_tensor(zacc2[:], zacc2[:], e2[:],
                                                ALU.add)
                    if c8 == 1 and pend[0] is not None:
                        pend[0][0]()
                    if c8 == 3 and pend[0] is not None:
                        pend[0][1]()
                        pend[0] = None
                    if qnext is not None:
                        qnext(16 * sb + c8)
                zfold = zp.tile([HD, NB], F16, tag="zf", bufs=1)
                nc.vector.tensor_tensor(zfold[:], zacc2[:, 0, :],
                                        zacc2[:, 1, :], ALU.add)
                pend[0] = make_z(pB, j, sb, zfold, PA)
            scope.__exit__(None, None, None)

        with ExitStack() as cx:
            # ---- x-resident era: projections + attention heads 0-2 ----
            px = cx.enter_context(tc.tile_pool(name="px", bufs=1))
            wkt = px.tile([HD, NDC, DG], F16, tag="wkt")    # 16KB/p
            xres = px.tile([HD, NDC, S], F16, tag="xres")   # 64KB/p
            wvt = px.tile([HD, NDC, DG], F16, tag="wvt")
            wqt = px.tile([HD, NDC, DG], F16, tag="wqt")

            # DMA order tuned so compute starts ASAP: wk, then x blocks
            # interleaved with wv/wq, cache K/V last.
            xr = xT.rearrange("(kc p) s -> p kc s", p=HD)
            nc.sync.dma_start(wkt[:], wk.rearrange("(kc p) n -> p kc n", p=HD))
            nc.sync.dma_start(xres[:, :, 0:NB], xr[:, :, 0:NB])
            nc.sync.dma_start(wvt[:], wv.rearrange("(kc p) n -> p kc n", p=HD))
            nc.sync.dma_start(xres[:, :, NB:2 * NB], xr[:, :, NB:2 * NB])
            nc.sync.dma_start(wqt[:], wq.rearrange("(kc p) n -> p kc n", p=HD))
            nc.sync.dma_start(xres[:, :, 2 * NB:3 * NB], xr[:, :, 2 * NB:3 * NB])
            nc.sync.dma_start(cv_t[:], cv.rearrange("(ss p) d -> p ss d", p=HD))
            nc.sync.dma_start(xres[:, :, 3 * NB:4 * NB], xr[:, :, 3 * NB:4 * NB])
            nc.sync.dma_start(ckT_t[:], ckT.rearrange("(m p) s -> p m s", p=HD))

            with tc.tile_pool(name="pA", bufs=1, space="PSUM") as pA:
                # ---- k-pass (seq-block-major, streams behind the x DMA) ----
                kscope = nc.named_scope("kp")
                kscope.__enter__()
                for sb in range(4):
                    for m in range(GH):
                        psk = pA.tile([HD, NB], F32, tag=f"kq{m % 2}",
                                      name="psk")
                        for kc in range(NDC):
                            nc.tensor.matmul(
                                psk[:], wkt[:, kc, HD * m:HD * (m + 1)],
                                xres[:, kc, NB * sb:NB * (sb + 1)],
                                start=(kc == 0), stop=(kc == NDC - 1))
                        nc.scalar.activation(kTn[:, m, NB * sb:NB * (sb + 1)],
                                             psk[:], AF.Identity,
                                             bias=bk_t[:, m:m + 1])
                kscope.__exit__(None, None, None)

                # ---- v-pass with head-0 q projection interleaved ----
                vscope = nc.named_scope("vp")
                vscope.__enter__()
                q0 = make_qproj(0, pA, wqt, xres)
                qi = 0
                for ss in range(S // HD):
                    psv = pA.tile([HD, DG], F32, tag=f"psv{ss % 2}", name="psv")
                    for kc in range(NDC):
                        nc.tensor.matmul(psv[:],
                                         xres[:, kc, HD * ss:HD * (ss + 1)],
                                         wvt[:, kc, :],
                                         start=(kc == 0), stop=False)
                    nc.tensor.matmul(psv[:], ones_r16[:], bv_t[:],
                                     start=False, stop=True)
                    nc.any.tensor_copy(vn_t[:, ss, :], psv[:])
                    for _ in range(4):
                        q0(qi)
                        qi += 1
                vscope.__exit__(None, None, None)

            # ---- attention heads 0-2, next head's q proj interleaved ----
            pBs = ExitStack()
            pB = pBs.enter_context(tc.tile_pool(name="pB", bufs=1, space="PSUM"))
            att_head(pB, 0, make_qproj(1, pB, wqt, xres))
            att_head(pB, 1, make_qproj(2, pB, wqt, xres))
            att_head(pB, 2, make_qproj(3, pB, wqt, xres))

        # ---- x freed; load phase-3 operands, run head 3, then out-proj ----
        ltp = ctx.enter_context(tc.tile_pool(name="ltp", bufs=1))
        wop = ctx.enter_context(tc.tile_pool(name="wop", bufs=1))
        lt_pool[0] = ltp
        wot = wop.tile([HD, 16, NB], F16, tag="wo")
        nc.sync.dma_start(wot[:], wo.rearrange("(c p) n -> p c n", p=HD))
        for j in range(GH - 1):
            for sb in range(4):
                if pend[0] is not None and (j, sb) == (GH - 2, 3):
                    # (2,3)'s gather is still pending (emitted inside head
                    # 3's stream); its lt load is emitted there too.
                    continue
                lt = ltp.tile([HD, 4, NB], F16, tag=f"lt{j}_{sb}",
                              name=f"lt{j}_{sb}")
                nc.sync.dma_start(lt[:],
                                  bounce_out[j][sb].rearrange("r p n -> p r n"))
                lts[(j, sb)] = lt

        att_head(pB, 3, None)
        emit_pending()          # z-finalize + gather for (3,3)
        pBs.close()

        # ---- phase 3: output projection ----
        p3scope = nc.named_scope("p3")
        p3scope.__enter__()
        with tc.tile_pool(name="pC", bufs=1, space="PSUM") as pC, \
             tc.tile_pool(name="p3", bufs=3) as p3:
            for m in range(S // HD):
                sb3, o = divmod(m, 4)
                psO = pC.tile([HD, NB], F32, tag=f"psO{m % 2}", name="psO")
                for j in range(GH):
                    for r in range(4):
                        nc.tensor.matmul(
                            psO[:], lts[(j, sb3)][:, r, HD * o:HD * (o + 1)],
                            wot[:, 4 * j + r, :],
                            start=(j == 0 and r == 0), stop=False,
                            skip_group_check=True)
                nc.tensor.matmul(psO[:], ones_r16[:], bo_t[:],
                                 start=False, stop=True, skip_group_check=True)
                ot = p3.tile([HD, NB], F32, tag="ot")
                nc.any.tensor_copy(ot[:], psO[:])
                nc.sync.dma_start(y[HD * m:HD * (m + 1), :], ot[:])
        p3scope.__exit__(None, None, None)

    nc.compile()
    return nc


_BUILT = None


def get_built():
    global _BUILT
    if _BUILT is None:
        _BUILT = build()
    return _BUILT


def make_in_maps(x, cache_k, cache_v, wq, bq, wk, bk, wv, bv, wo, bo):
    x = np.asarray(x)
    cache_k = np.asarray(cache_k)
    cache_v = np.asarray(cache_v)
    wq, bq = np.asarray(wq), np.asarray(bq)
    wk, bk = np.asarray(wk), np.asarray(bk)
    wv, bv = np.asarray(wv), np.asarray(bv)
    wo, bo = np.asarray(wo), np.asarray(bo)

    # permute wo rows to match gather order: lhsT chunk jr=(4j+r) holds head 4r+j
    perm = np.concatenate([
        np.arange(HD * (4 * r + j), HD * (4 * r + j) + HD)
        for j in range(GH) for r in range(4)
    ])
    wo_p = wo[perm, :]

    in_maps = []
    for c in range(8):
        b, g = divmod(c, 4)
        sl = slice(DG * g, DG * (g + 1))
        in_maps.append({
            "xT": np.ascontiguousarray(x[b].T).astype(np.float16),
            "wq": wq[:, sl].astype(np.float16),
            "bq": (bq[sl] * INV_SQRT_HD).astype(np.float16),
            "wk": wk[:, sl].astype(np.float16),
            "bk": bk[sl].astype(np.float16),
            "wv": wv[:, sl].astype(np.float16),
            "bv": bv[sl].astype(np.float16),
            "ckT": np.ascontiguousarray(cache_k[b][:, sl].T).astype(np.float16),
            "cv": cache_v[b][:, sl].astype(np.float16),
            "wo": wo_p[:, sl].astype(np.float16),
            "bo": bo[sl].astype(np.float16),
        })
    return in_maps


def assemble(results):
    out = np.empty((B, S, D), np.float32)
    for c in range(8):
        b, g = divmod(c, 4)
        out[b, :, DG * g:DG * (g + 1)] = results[c]["y"]
    return out


def kernel(**inputs):
    nc = get_built()
    in_maps = make_in_maps(**inputs)
    res = run_bass_kernel_spmd(nc, in_maps, core_ids=list(range(8)))
    return assemble(res.results)


# revision 7
# speedup vs baseline: 1.1964x; 1.0174x over previous
"""Cached self-attention Trainium2 kernel (v6).

Sharding: 8 cores = 2 batches x 4 head-groups. Core c: batch b=c//4, group
g=c%4 owns heads 4g..4g+3 (columns 512g:512g+512 of the q/k/v projections).

v6 vs v5 (637us):
- softmax-Z finalization entirely off TensorE: GpSimd partition_all_reduce
  replaces the ones-matmul partition reduce, and the reciprocal is taken
  on the broadcast [128,512] result directly (no broadcast matmul, no z
  PSUM bank, no deferred emission machinery).
- bias adds via a precomputed broadcast-bias SBUF tile + DVE add during
  PSUM evacuation (removes the 32 ones-matmul bias adds for v/o).
- psO 4 PSUM banks (was 2: each m-tile stalled ~0.8us on its evac).
- phase-3/v evacuation on DVE explicitly (ScalarE queue blocks on y-DMA
  waits otherwise).
- DMA order: wk, then all 4 x blocks, then wv/wq/cache (x gates k-pass).
- PA 3 PSUM banks. PSUM: pss0(2) pss1(2) PA(3) psq(1) = 8 banks.

Measured v5 facts this builds on: the PE runs at 13/16 throttle (~1.95
GHz) under sustained load, so the matmul stream floor is ~262ns/MM; LDW
overlaps MMs fine; exp on ScalarE (FD=1024) hides under the 5-MM c8
group.
"""
import numpy as np
from contextlib import ExitStack

import concourse.bass as bass
import concourse.tile as tile
from concourse import bacc, mybir
from concourse.bass_isa import ReduceOp
from concourse.bass_utils import run_bass_kernel_spmd

B, S, PC, D, H = 2, 2048, 2048, 2048, 16
HD = D // H            # 128 head dim
GH = H // 4            # 4 heads per core
DG = GH * HD           # 512 head-dims per core
NB = 512               # seq block size
NDC = D // HD          # 16 contraction chunks
NCC = PC // HD         # 16 cache key chunks
NKC = (PC + S) // HD   # 32 total key chunks
F16 = mybir.dt.float16
F32 = mybir.dt.float32
AF = mybir.ActivationFunctionType
ALU = mybir.AluOpType
INV_SQRT_HD = float(1.0 / np.sqrt(HD))

GROUPS = [[0, 1, 2, 3], [4, 5, 6, 7]]


def build():
    nc = bacc.Bacc("TRN2", target_bir_lowering=False, debug=False, num_devices=8)

    def inp(name, shape):
        return nc.dram_tensor(name, shape, F16, kind="ExternalInput").ap()

    xT = inp("xT", [D, S])          # x[b].T
    wq = inp("wq", [D, DG])         # wq[:, 512g:512g+512]
    bq = inp("bq", [DG])            # bq slice / sqrt(HD)
    wk = inp("wk", [D, DG])
    bk = inp("bk", [DG])
    wv = inp("wv", [D, DG])
    bv = inp("bv", [DG])
    ckT = inp("ckT", [DG, PC])      # cache_k[b,:,slice].T
    cv = inp("cv", [PC, DG])        # cache_v[b,:,slice]
    wo = inp("wo", [D, DG])         # wo rows permuted to gather order, cols sliced
    bo = inp("bo", [DG])
    y = nc.dram_tensor("y", [S, DG], F32, kind="ExternalOutput").ap()

    with tile.TileContext(nc) as tc, ExitStack() as ctx:
        res = ctx.enter_context(tc.tile_pool(name="res", bufs=1))
        dram = ctx.enter_context(tc.tile_pool(name="dram", bufs=1, space="DRAM"))

        # tiny whole-kernel residents (issued first: needed by early evacs)
        bq_t = res.tile([HD, GH], F16, tag="bq")
        bk_t = res.tile([HD, GH], F16, tag="bk")
        bv_t = res.tile([1, DG], F16, tag="bv")
        bo_t = res.tile([1, DG], F16, tag="bo")
        ones_r16 = res.tile([1, HD], F16, tag="ones_r16")  # [1,128] ones
        bias_bc = res.tile([HD, DG], F16, tag="bias_bc")   # bv then bo, bcast
        nc.sync.dma_start(bq_t[:], bq.rearrange("(m p) -> p m", p=HD))
        nc.sync.dma_start(bk_t[:], bk.rearrange("(m p) -> p m", p=HD))
        nc.sync.dma_start(bv_t[:], bv[None, :])
        nc.sync.dma_start(bo_t[:], bo[None, :])
        nc.vector.memset(ones_r16[:], 1.0)

        # big residents for phases 1+2
        kTn = res.tile([HD, GH, S], F16, tag="kTn")         # 16KB/p
        vn_t = res.tile([HD, S // HD, DG], F16, tag="vn")   # 16KB/p
        cv_t = res.tile([HD, NCC, DG], F16, tag="cv")       # 16KB/p
        ckT_t = res.tile([HD, GH, PC], F16, tag="ckT")      # 16KB/p

        # attention working pools (whole-kernel scope)
        qp = ctx.enter_context(tc.tile_pool(name="qp", bufs=2))
        ep = ctx.enter_context(tc.tile_pool(name="ep", bufs=3))
        zp = ctx.enter_context(tc.tile_pool(name="zp", bufs=1))
        apool = ctx.enter_context(tc.tile_pool(name="apool", bufs=2))

        # collective bounce buffers, per (head, seq-block)
        bounce_in = [[dram.tile([HD, NB], F16, tag=f"bi{j}_{sb}",
                                name=f"bi{j}_{sb}") for sb in range(4)]
                     for j in range(GH)]
        bounce_out = [[dram.tile([4, HD, NB], F16, tag=f"bg{j}_{sb}",
                                 name=f"bg{j}_{sb}") for sb in range(4)]
                      for j in range(GH)]

        qT_tiles = {}
        lts = {}
        lt_pool = [None]       # set once the phase-3 lt pool is open

        def make_qproj(jq, pool, wqt, xres):
            qt = qp.tile([HD, S], F16, tag="qT", name=f"qT{jq}")
            qT_tiles[jq] = qt
            cur = {}

            def step(s):
                sbq, kc = divmod(s, NDC)
                if kc == 0:
                    cur["psq"] = pool.tile([HD, NB], F32, tag="psq", name="psq")
                psq = cur["psq"]
                nc.tensor.matmul(psq[:], wqt[:, kc, HD * jq:HD * (jq + 1)],
                                 xres[:, kc, NB * sbq:NB * (sbq + 1)],
                                 start=(kc == 0), stop=(kc == NDC - 1),
                                 skip_group_check=True)
                if kc == NDC - 1:
                    nc.scalar.activation(qt[:, NB * sbq:NB * (sbq + 1)], psq[:],
                                         AF.Identity, bias=bq_t[:, jq:jq + 1],
                                         scale=INV_SQRT_HD)
            return step

        def att_head(pB, j, qnext):
            scope = nc.named_scope(f"h{j}")
            scope.__enter__()
            for sb in range(4):
                PA = pB.tile([HD, NB], F32, tag=f"PA{sb % 3}", name="PA")
                zacc2 = zp.tile([HD, 2, NB], F16, tag="za")
                qTs = qT_tiles[j][:, NB * sb:NB * (sb + 1)]
                for c8 in range(NKC // 2):
                    pss = pB.tile([HD, 2, NB], F32, tag=f"pss{c8 % 2}",
                                  name="pss")
                    for i in range(2):
                        c = 2 * c8 + i
                        if c < NCC:
                            kt = ckT_t[:, j, HD * c:HD * (c + 1)]
                        else:
                            kt = kTn[:, j, HD * (c - NCC):HD * (c - NCC + 1)]
                        nc.tensor.matmul(pss[:, i, :], kt, qTs,
                                         start=True, stop=True)
                    e2 = ep.tile([HD, 2, NB], F16, tag="e2")
                    nc.scalar.activation(e2[:], pss[:], AF.Exp)
                    for i in range(2):
                        c = 2 * c8 + i
                        if c < NCC:
                            vt = cv_t[:, c, HD * j:HD * (j + 1)]
                        else:
                            vt = vn_t[:, c - NCC, HD * j:HD * (j + 1)]
                        nc.tensor.matmul(PA[:], vt, e2[:, i, :],
                                         start=(c == 0), stop=(c == NKC - 1),
                                         skip_group_check=True)
                    if c8 == 0:
                        nc.vector.tensor_copy(zacc2[:], e2[:])
                    else:
                        nc.vector.tensor_tensor(zacc2[:], zacc2[:], e2[:],
                                                ALU.add)
                    if qnext is not None:
                        qnext(16 * sb + c8)
                # Z finalize + normalize + gather: DVE/GpSimd only, TensorE
                # stream flows uninterrupted into the next block.
                zfold = zp.tile([HD, NB], F16, tag="zf")
                nc.vector.tensor_tensor(zfold[:], zacc2[:, 0, :],
                                        zacc2[:, 1, :], ALU.add)
                zsum = zp.tile([HD, NB], F32, tag="zs")
                nc.gpsimd.partition_all_reduce(zsum[:], zfold[:], HD,
                                               ReduceOp.add)
                zb = zp.tile([HD, NB], F32, tag="zb")
                nc.vector.reciprocal_approx_fast(zb[:], zsum[:])
                ahead = apool.tile([HD, NB], F16, tag="ah")
                nc.vector.tensor_tensor(ahead[:], PA[:], zb[:], ALU.mult)
                nc.sync.dma_start(bounce_in[j][sb][:], ahead[:])
                nc.gpsimd.collective_compute(
                    "AllGather", ALU.bypass, replica_groups=GROUPS,
                    ins=[bounce_in[j][sb].opt()], outs=[bounce_out[j][sb].opt()])
                if lt_pool[0] is not None:
                    lt = lt_pool[0].tile([HD, 4, NB], F16, tag=f"lt{j}_{sb}",
                                         name=f"lt{j}_{sb}")
                    nc.sync.dma_start(
                        lt[:], bounce_out[j][sb].rearrange("r p n -> p r n"))
                    lts[(j, sb)] = lt
            scope.__exit__(None, None, None)

        with ExitStack() as cx:
            # ---- x-resident era: projections + attention heads 0-2 ----
            px = cx.enter_context(tc.tile_pool(name="px", bufs=1))
            wkt = px.tile([HD, NDC, DG], F16, tag="wkt")    # 16KB/p
            xres = px.tile([HD, NDC, S], F16, tag="xres")   # 64KB/p
            wvt = px.tile([HD, NDC, DG], F16, tag="wvt")
            wqt = px.tile([HD, NDC, DG], F16, tag="wqt")

            # DMA order tuned so compute starts ASAP.
            xr = xT.rearrange("(kc p) s -> p kc s", p=HD)
            nc.sync.dma_start(wkt[:], wk.rearrange("(kc p) n -> p kc n", p=HD))
            for sb in range(4):
                nc.sync.dma_start(xres[:, :, NB * sb:NB * (sb + 1)],
                                  xr[:, :, NB * sb:NB * (sb + 1)])
            nc.sync.dma_start(wvt[:], wv.rearrange("(kc p) n -> p kc n", p=HD))
            nc.sync.dma_start(wqt[:], wq.rearrange("(kc p) n -> p kc n", p=HD))
            nc.sync.dma_start(cv_t[:], cv.rearrange("(ss p) d -> p ss d", p=HD))
            nc.sync.dma_start(ckT_t[:], ckT.rearrange("(m p) s -> p m s", p=HD))

            with tc.tile_pool(name="pA", bufs=1, space="PSUM") as pA:
                # broadcast bv across partitions once (zero-cost bias adds on
                # the v evacuations)
                psb0 = pA.tile([HD, DG], F32, tag="psv0", name="psb0")
                nc.tensor.matmul(psb0[:], ones_r16[:], bv_t[:],
                                 start=True, stop=True)
                nc.vector.tensor_copy(bias_bc[:], psb0[:])

                # ---- k-pass (seq-block-major, streams behind the x DMA) ----
                kscope = nc.named_scope("kp")
                kscope.__enter__()
                for sb in range(4):
                    for m in range(GH):
                        psk = pA.tile([HD, NB], F32, tag=f"kq{m % 2}",
                                      name="psk")
                        for kc in range(NDC):
                            nc.tensor.matmul(
                                psk[:], wkt[:, kc, HD * m:HD * (m + 1)],
                                xres[:, kc, NB * sb:NB * (sb + 1)],
                                start=(kc == 0), stop=(kc == NDC - 1))
                        nc.scalar.activation(kTn[:, m, NB * sb:NB * (sb + 1)],
                                             psk[:], AF.Identity,
                                             bias=bk_t[:, m:m + 1])
                kscope.__exit__(None, None, None)

                # ---- v-pass with head-0 q projection interleaved ----
                vscope = nc.named_scope("vp")
                vscope.__enter__()
                q0 = make_qproj(0, pA, wqt, xres)
                qi = 0
                for ss in range(S // HD):
                    psv = pA.tile([HD, DG], F32, tag=f"psv{ss % 2}", name="psv")
                    for kc in range(NDC):
                        nc.tensor.matmul(psv[:],
                                         xres[:, kc, HD * ss:HD * (ss + 1)],
                                         wvt[:, kc, :],
                                         start=(kc == 0),
                                         stop=(kc == NDC - 1))
                    nc.vector.tensor_tensor(vn_t[:, ss, :], psv[:], bias_bc[:],
                                            ALU.add)
                    for _ in range(4):
                        q0(qi)
                        qi += 1
                # rewrite bias_bc with broadcast bo for phase 3
                psb1 = pA.tile([HD, DG], F32, tag="psv1", name="psb1")
                nc.tensor.matmul(psb1[:], ones_r16[:], bo_t[:],
                                 start=True, stop=True)
                nc.vector.tensor_copy(bias_bc[:], psb1[:])
                vscope.__exit__(None, None, None)

            # ---- attention heads 0-2, next head's q proj interleaved ----
            pBs = ExitStack()
            pB = pBs.enter_context(tc.tile_pool(name="pB", bufs=1, space="PSUM"))
            att_head(pB, 0, make_qproj(1, pB, wqt, xres))
            att_head(pB, 1, make_qproj(2, pB, wqt, xres))
            att_head(pB, 2, make_qproj(3, pB, wqt, xres))

        # ---- x freed; load phase-3 operands, run head 3, then out-proj ----
        ltp = ctx.enter_context(tc.tile_pool(name="ltp", bufs=1))
        wop = ctx.enter_context(tc.tile_pool(name="wop", bufs=1))
        lt_pool[0] = ltp
        wot = wop.tile([HD, 16, NB], F16, tag="wo")
        nc.sync.dma_start(wot[:], wo.rearrange("(c p) n -> p c n", p=HD))
        for j in range(GH - 1):
            for sb in range(4):
                lt = ltp.tile([HD, 4, NB], F16, tag=f"lt{j}_{sb}",
                              name=f"lt{j}_{sb}")
                nc.sync.dma_start(lt[:],
                                  bounce_out[j][sb].rearrange("r p n -> p r n"))
                lts[(j, sb)] = lt

        att_head(pB, 3, None)
        pBs.close()

        # ---- phase 3: output projection ----
        p3scope = nc.named_scope("p3")
        p3scope.__enter__()
        with tc.tile_pool(name="pC", bufs=1, space="PSUM") as pC, \
             tc.tile_pool(name="p3", bufs=4) as p3:
            for m in range(S // HD):
                sb3, o = divmod(m, 4)
                psO = pC.tile([HD, NB], F32, tag=f"psO{m % 4}", name="psO")
                for j in range(GH):
                    for r in range(4):
                        nc.tensor.matmul(
                            psO[:], lts[(j, sb3)][:, r, HD * o:HD * (o + 1)],
                            wot[:, 4 * j + r, :],
                            start=(j == 0 and r == 0),
                            stop=(j == GH - 1 and r == 3),
                            skip_group_check=True)
                ot = p3.tile([HD, NB], F32, tag="ot")
                nc.vector.tensor_tensor(ot[:], psO[:], bias_bc[:], ALU.add)
                nc.sync.dma_start(y[HD * m:HD * (m + 1), :], ot[:])
        p3scope.__exit__(None, None, None)

    nc.compile()
    return nc


_BUILT = None


def get_built():
    global _BUILT
    if _BUILT is None:
        _BUILT = build()
    return _BUILT


def make_in_maps(x, cache_k, cache_v, wq, bq, wk, bk, wv, bv, wo, bo):
    x = np.asarray(x)
    cache_k = np.asarray(cache_k)
    cache_v = np.asarray(cache_v)
    wq, bq = np.asarray(wq), np.asarray(bq)
    wk, bk = np.asarray(wk), np.asarray(bk)
    wv, bv = np.asarray(wv), np.asarray(bv)
    wo, bo = np.asarray(wo), np.asarray(bo)

    # permute wo rows to match gather order: lhsT chunk jr=(4j+r) holds head 4r+j
    perm = np.concatenate([
        np.arange(HD * (4 * r + j), HD * (4 * r + j) + HD)
        for j in range(GH) for r in range(4)
    ])
    wo_p = wo[perm, :]

    in_maps = []
    for c in range(8):
        b, g = divmod(c, 4)
        sl = slice(DG * g, DG * (g + 1))
        in_maps.append({
            "xT": np.ascontiguousarray(x[b].T).astype(np.float16),
            "wq": wq[:, sl].astype(np.float16),
            "bq": (bq[sl] * INV_SQRT_HD).astype(np.float16),
            "wk": wk[:, sl].astype(np.float16),
            "bk": bk[sl].astype(np.float16),
            "wv": wv[:, sl].astype(np.float16),
            "bv": bv[sl].astype(np.float16),
            "ckT": np.ascontiguousarray(cache_k[b][:, sl].T).astype(np.float16),
            "cv": cache_v[b][:, sl].astype(np.float16),
            "wo": wo_p[:, sl].astype(np.float16),
            "bo": bo[sl].astype(np.float16),
        })
    return in_maps


def assemble(results):
    out = np.empty((B, S, D), np.float32)
    for c in range(8):
        b, g = divmod(c, 4)
        out[b, :, DG * g:DG * (g + 1)] = results[c]["y"]
    return out


def kernel(**inputs):
    nc = get_built()
    in_maps = make_in_maps(**inputs)
    res = run_bass_kernel_spmd(nc, in_maps, core_ids=list(range(8)))
    return assemble(res.results)


# revision 11
# speedup vs baseline: 1.2149x; 1.0155x over previous
"""Cached self-attention Trainium2 kernel (v6).

Sharding: 8 cores = 2 batches x 4 head-groups. Core c: batch b=c//4, group
g=c%4 owns heads 4g..4g+3 (columns 512g:512g+512 of the q/k/v projections).

v6 vs v5 (637us):
- softmax-Z finalization entirely off TensorE: GpSimd partition_all_reduce
  replaces the ones-matmul partition reduce, and the reciprocal is taken
  on the broadcast [128,512] result directly (no broadcast matmul, no z
  PSUM bank, no deferred emission machinery).
- bias adds via a precomputed broadcast-bias SBUF tile + DVE add during
  PSUM evacuation (removes the 32 ones-matmul bias adds for v/o).
- psO 4 PSUM banks (was 2: each m-tile stalled ~0.8us on its evac).
- phase-3/v evacuation on DVE explicitly (ScalarE queue blocks on y-DMA
  waits otherwise).
- DMA order: wk, then all 4 x blocks, then wv/wq/cache (x gates k-pass).
- PA 3 PSUM banks. PSUM: pss0(2) pss1(2) PA(3) psq(1) = 8 banks.

Measured v5 facts this builds on: the PE runs at 13/16 throttle (~1.95
GHz) under sustained load, so the matmul stream floor is ~262ns/MM; LDW
overlaps MMs fine; exp on ScalarE (FD=1024) hides under the 5-MM c8
group.
"""
import numpy as np
from contextlib import ExitStack

import concourse.bass as bass
import concourse.tile as tile
from concourse import bacc, mybir
from concourse.bass_isa import ReduceOp
from concourse.bass_utils import run_bass_kernel_spmd

B, S, PC, D, H = 2, 2048, 2048, 2048, 16
HD = D // H            # 128 head dim
GH = H // 4            # 4 heads per core
DG = GH * HD           # 512 head-dims per core
NB = 512               # seq block size
NDC = D // HD          # 16 contraction chunks
NCC = PC // HD         # 16 cache key chunks
NKC = (PC + S) // HD   # 32 total key chunks
F16 = mybir.dt.float16
F32 = mybir.dt.float32
AF = mybir.ActivationFunctionType
ALU = mybir.AluOpType
INV_SQRT_HD = float(1.0 / np.sqrt(HD))

GROUPS = [[0, 1, 2, 3], [4, 5, 6, 7]]


def build():
    nc = bacc.Bacc("TRN2", target_bir_lowering=False, debug=False, num_devices=8)

    def inp(name, shape):
        return nc.dram_tensor(name, shape, F16, kind="ExternalInput").ap()

    xT = inp("xT", [D, S])          # x[b].T
    wq = inp("wq", [D, DG])         # wq[:, 512g:512g+512]
    bq = inp("bq", [DG])            # bq slice / sqrt(HD)
    wk = inp("wk", [D, DG])
    bk = inp("bk", [DG])
    wv = inp("wv", [D, DG])
    bv = inp("bv", [DG])
    ckT = inp("ckT", [DG, PC])      # cache_k[b,:,slice].T
    cv = inp("cv", [PC, DG])        # cache_v[b,:,slice]
    wo = inp("wo", [D, DG])         # wo rows permuted to gather order, cols sliced
    bo = inp("bo", [DG])
    y = nc.dram_tensor("y", [S, DG], F32, kind="ExternalOutput").ap()

    with tile.TileContext(nc) as tc, ExitStack() as ctx:
        res = ctx.enter_context(tc.tile_pool(name="res", bufs=1))
        dram = ctx.enter_context(tc.tile_pool(name="dram", bufs=1, space="DRAM"))

        # tiny whole-kernel residents (issued first: needed by early evacs)
        bq_t = res.tile([HD, GH], F16, tag="bq")
        bk_t = res.tile([HD, GH], F16, tag="bk")
        bv_t = res.tile([1, DG], F16, tag="bv")
        bo_t = res.tile([1, DG], F16, tag="bo")
        ones_r16 = res.tile([1, HD], F16, tag="ones_r16")  # [1,128] ones
        bias_bc = res.tile([HD, DG], F16, tag="bias_bc")   # bv then bo, bcast
        nc.sync.dma_start(bq_t[:], bq.rearrange("(m p) -> p m", p=HD))
        nc.sync.dma_start(bk_t[:], bk.rearrange("(m p) -> p m", p=HD))
        nc.sync.dma_start(bv_t[:], bv[None, :])
        nc.sync.dma_start(bo_t[:], bo[None, :])
        nc.vector.memset(ones_r16[:], 1.0)

        # big residents for phases 1+2
        kTn = res.tile([HD, GH, S], F16, tag="kTn")         # 16KB/p
        vn_t = res.tile([HD, S // HD, DG], F16, tag="vn")   # 16KB/p
        cv_t = res.tile([HD, NCC, DG], F16, tag="cv")       # 16KB/p
        ckT_t = res.tile([HD, GH, PC], F16, tag="ckT")      # 16KB/p

        # attention working pools (whole-kernel scope)
        qp = ctx.enter_context(tc.tile_pool(name="qp", bufs=2))
        ep = ctx.enter_context(tc.tile_pool(name="ep", bufs=3))
        zp = ctx.enter_context(tc.tile_pool(name="zp", bufs=1))
        apool = ctx.enter_context(tc.tile_pool(name="apool", bufs=2))

        # collective bounce buffers, per (head, seq-block)
        bounce_in = [[dram.tile([HD, NB], F16, tag=f"bi{j}_{sb}",
                                name=f"bi{j}_{sb}") for sb in range(4)]
                     for j in range(GH)]
        bounce_out = [[dram.tile([4, HD, NB], F16, tag=f"bg{j}_{sb}",
                                 name=f"bg{j}_{sb}") for sb in range(4)]
                      for j in range(GH)]

        qT_tiles = {}
        lts = {}
        lt_pool = [None]       # set once the phase-3 lt pool is open
        pend = [None]          # deferred normalize+gather for previous block

        def make_fin(j, sb, zsum, PA):
            # reciprocal + normalize + gather for block (j,sb); emitted ~4
            # c8-iterations into the NEXT block so the DVE queue never
            # blocks on the GpSimd all-reduce (which itself can queue
            # behind a peer-skewed collective).
            def go():
                zb = zp.tile([HD, NB], F32, tag="zb")
                nc.vector.reciprocal_approx_fast(zb[:], zsum[:])
                ahead = apool.tile([HD, NB], F16, tag="ah")
                nc.vector.tensor_tensor(ahead[:], PA[:], zb[:], ALU.mult)
                nc.sync.dma_start(bounce_in[j][sb][:], ahead[:])
                nc.gpsimd.collective_compute(
                    "AllGather", ALU.bypass, replica_groups=GROUPS,
                    ins=[bounce_in[j][sb].opt()], outs=[bounce_out[j][sb].opt()])
                if lt_pool[0] is not None:
                    lt = lt_pool[0].tile([HD, 4, NB], F16, tag=f"lt{j}_{sb}",
                                         name=f"lt{j}_{sb}")
                    nc.sync.dma_start(
                        lt[:], bounce_out[j][sb].rearrange("r p n -> p r n"))
                    lts[(j, sb)] = lt
            return go

        def emit_pending():
            if pend[0] is not None:
                pend[0]()
                pend[0] = None

        def make_qproj(jq, pool, wqt, xres):
            qt = qp.tile([HD, S], F16, tag="qT", name=f"qT{jq}")
            qT_tiles[jq] = qt
            cur = {}

            def step(s):
                sbq, kc = divmod(s, NDC)
                if kc == 0:
                    cur["psq"] = pool.tile([HD, NB], F32, tag="psq", name="psq")
                psq = cur["psq"]
                nc.tensor.matmul(psq[:], wqt[:, kc, HD * jq:HD * (jq + 1)],
                                 xres[:, kc, NB * sbq:NB * (sbq + 1)],
                                 start=(kc == 0), stop=(kc == NDC - 1),
                                 skip_group_check=True)
                if kc == NDC - 1:
                    nc.scalar.activation(qt[:, NB * sbq:NB * (sbq + 1)], psq[:],
                                         AF.Identity, bias=bq_t[:, jq:jq + 1],
                                         scale=INV_SQRT_HD)
            return step

        def att_head(pB, j, qnext):
            scope = nc.named_scope(f"h{j}")
            scope.__enter__()
            for sb in range(4):
                PA = pB.tile([HD, NB], F32, tag=f"PA{sb % 3}", name="PA")
                zacc2 = zp.tile([HD, 2, NB], F16, tag="za")
                qTs = qT_tiles[j][:, NB * sb:NB * (sb + 1)]
                for c8 in range(NKC // 2):
                    pss = pB.tile([HD, 2, NB], F32, tag=f"pss{c8 % 2}",
                                  name="pss")
                    for i in range(2):
                        c = 2 * c8 + i
                        if c < NCC:
                            kt = ckT_t[:, j, HD * c:HD * (c + 1)]
                        else:
                            kt = kTn[:, j, HD * (c - NCC):HD * (c - NCC + 1)]
                        nc.tensor.matmul(pss[:, i, :], kt, qTs,
                                         start=True, stop=True)
                    e2 = ep.tile([HD, 2, NB], F16, tag="e2")
                    nc.scalar.activation(e2[:], pss[:], AF.Exp)
                    for i in range(2):
                        c = 2 * c8 + i
                        if c < NCC:
                            vt = cv_t[:, c, HD * j:HD * (j + 1)]
                        else:
                            vt = vn_t[:, c - NCC, HD * j:HD * (j + 1)]
                        nc.tensor.matmul(PA[:], vt, e2[:, i, :],
                                         start=(c == 0), stop=(c == NKC - 1),
                                         skip_group_check=True)
                    if c8 == 0:
                        nc.vector.tensor_copy(zacc2[:], e2[:])
                    else:
                        nc.vector.tensor_tensor(zacc2[:], zacc2[:], e2[:],
                                                ALU.add)
                    if c8 == 4:
                        emit_pending()
                    if qnext is not None:
                        qnext(16 * sb + c8)
                # Z partition reduce: DVE fold + GpSimd all-reduce, inline;
                # the consumer chain is deferred into the next block.
                zfold = zp.tile([HD, NB], F16, tag="zf")
                nc.vector.tensor_tensor(zfold[:], zacc2[:, 0, :],
                                        zacc2[:, 1, :], ALU.add)
                zsum = zp.tile([HD, NB], F32, tag="zs")
                nc.gpsimd.partition_all_reduce(zsum[:], zfold[:], HD,
                                               ReduceOp.add)
                pend[0] = make_fin(j, sb, zsum, PA)
            scope.__exit__(None, None, None)

        with ExitStack() as cx:
            # ---- x-resident era: projections + attention heads 0-2 ----
            px = cx.enter_context(tc.tile_pool(name="px", bufs=1))
            wkt = px.tile([HD, NDC, DG], F16, tag="wkt")    # 16KB/p
            xres = px.tile([HD, NDC, S], F16, tag="xres")   # 64KB/p
            wvt = px.tile([HD, NDC, DG], F16, tag="wvt")
            wqt = px.tile([HD, NDC, DG], F16, tag="wqt")

            # DMA order tuned so compute starts ASAP.
            xr = xT.rearrange("(kc p) s -> p kc s", p=HD)
            wkr = wk.rearrange("(kc p) n -> p kc n", p=HD)
            nc.sync.dma_start(wkt[:, :, 0:HD], wkr[:, :, 0:HD])
            nc.sync.dma_start(xres[:, :, 0:NB], xr[:, :, 0:NB])
            nc.sync.dma_start(wkt[:, :, HD:DG], wkr[:, :, HD:DG])
            for sb in range(1, 4):
                nc.sync.dma_start(xres[:, :, NB * sb:NB * (sb + 1)],
                                  xr[:, :, NB * sb:NB * (sb + 1)])
            nc.sync.dma_start(wvt[:], wv.rearrange("(kc p) n -> p kc n", p=HD))
            nc.sync.dma_start(wqt[:], wq.rearrange("(kc p) n -> p kc n", p=HD))
            nc.sync.dma_start(cv_t[:], cv.rearrange("(ss p) d -> p ss d", p=HD))
            nc.sync.dma_start(ckT_t[:], ckT.rearrange("(m p) s -> p m s", p=HD))

            with tc.tile_pool(name="pA", bufs=1, space="PSUM") as pA:
                # broadcast bv across partitions once (zero-cost bias adds on
                # the v evacuations)
                psb0 = pA.tile([HD, DG], F32, tag="psv0", name="psb0")
                nc.tensor.matmul(psb0[:], ones_r16[:], bv_t[:],
                                 start=True, stop=True)
                nc.vector.tensor_copy(bias_bc[:], psb0[:])

                # ---- k-pass (seq-block-major, streams behind the x DMA) ----
                kscope = nc.named_scope("kp")
                kscope.__enter__()
                for sb in range(4):
                    for m in range(GH):
                        psk = pA.tile([HD, NB], F32, tag=f"kq{m % 2}",
                                      name="psk")
                        for kc in range(NDC):
                            nc.tensor.matmul(
                                psk[:], wkt[:, kc, HD * m:HD * (m + 1)],
                                xres[:, kc, NB * sb:NB * (sb + 1)],
                                start=(kc == 0), stop=(kc == NDC - 1))
                        nc.scalar.activation(kTn[:, m, NB * sb:NB * (sb + 1)],
                                             psk[:], AF.Identity,
                                             bias=bk_t[:, m:m + 1])
                kscope.__exit__(None, None, None)

                # ---- v-pass with head-0 q projection interleaved ----
                vscope = nc.named_scope("vp")
                vscope.__enter__()
                q0 = make_qproj(0, pA, wqt, xres)
                qi = 0
                for ss in range(S // HD):
                    psv = pA.tile([HD, DG], F32, tag=f"psv{ss % 2}", name="psv")
                    for kc in range(NDC):
                        nc.tensor.matmul(psv[:],
                                         xres[:, kc, HD * ss:HD * (ss + 1)],
                                         wvt[:, kc, :],
                                         start=(kc == 0),
                                         stop=(kc == NDC - 1))
                    nc.vector.tensor_tensor(vn_t[:, ss, :], psv[:], bias_bc[:],
                                            ALU.add)
                    for _ in range(4):
                        q0(qi)
                        qi += 1
                # rewrite bias_bc with broadcast bo for phase 3
                psb1 = pA.tile([HD, DG], F32, tag="psv1", name="psb1")
                nc.tensor.matmul(psb1[:], ones_r16[:], bo_t[:],
                                 start=True, stop=True)
                nc.vector.tensor_copy(bias_bc[:], psb1[:])
                vscope.__exit__(None, None, None)

            # ---- attention heads 0-2, next head's q proj interleaved ----
            pBs = ExitStack()
            pB = pBs.enter_context(tc.tile_pool(name="pB", bufs=1, space="PSUM"))
            att_head(pB, 0, make_qproj(1, pB, wqt, xres))
            att_head(pB, 1, make_qproj(2, pB, wqt, xres))
            att_head(pB, 2, make_qproj(3, pB, wqt, xres))

        # ---- x freed; load phase-3 operands, run head 3, then out-proj ----
        ltp = ctx.enter_context(tc.tile_pool(name="ltp", bufs=1))
        wop = ctx.enter_context(tc.tile_pool(name="wop", bufs=1))
        lt_pool[0] = ltp
        wot = wop.tile([HD, 16, NB], F16, tag="wo")
        nc.sync.dma_start(wot[:], wo.rearrange("(c p) n -> p c n", p=HD))
        for j in range(GH - 1):
            for sb in range(4):
                if pend[0] is not None and (j, sb) == (GH - 2, 3):
                    # (2,3)'s gather is still pending (deferred into head
                    # 3's stream); its lt load is emitted there too.
                    continue
                lt = ltp.tile([HD, 4, NB], F16, tag=f"lt{j}_{sb}",
                              name=f"lt{j}_{sb}")
                nc.sync.dma_start(lt[:],
                                  bounce_out[j][sb].rearrange("r p n -> p r n"))
                lts[(j, sb)] = lt

        att_head(pB, 3, None)
        emit_pending()          # normalize + gather for (3,3)
        pBs.close()

        # ---- phase 3: output projection ----
        p3scope = nc.named_scope("p3")
        p3scope.__enter__()
        with tc.tile_pool(name="pC", bufs=1, space="PSUM") as pC, \
             tc.tile_pool(name="p3", bufs=4) as p3:
            for m in range(S // HD):
                sb3, o = divmod(m, 4)
                psO = pC.tile([HD, NB], F32, tag=f"psO{m % 4}", name="psO")
                for j in range(GH):
                    for r in range(4):
                        nc.tensor.matmul(
                            psO[:], lts[(j, sb3)][:, r, HD * o:HD * (o + 1)],
                            wot[:, 4 * j + r, :],
                            start=(j == 0 and r == 0),
                            stop=(j == GH - 1 and r == 3),
                            skip_group_check=True)
                ot = p3.tile([HD, NB], F32, tag="ot")
                nc.vector.tensor_tensor(ot[:], psO[:], bias_bc[:], ALU.add)
                nc.sync.dma_start(y[HD * m:HD * (m + 1), :], ot[:])
        p3scope.__exit__(None, None, None)

    nc.compile()
    return nc


_BUILT = None


def get_built():
    global _BUILT
    if _BUILT is None:
        _BUILT = build()
    return _BUILT


def make_in_maps(x, cache_k, cache_v, wq, bq, wk, bk, wv, bv, wo, bo):
    x = np.asarray(x)
    cache_k = np.asarray(cache_k)
    cache_v = np.asarray(cache_v)
    wq, bq = np.asarray(wq), np.asarray(bq)
    wk, bk = np.asarray(wk), np.asarray(bk)
    wv, bv = np.asarray(wv), np.asarray(bv)
    wo, bo = np.asarray(wo), np.asarray(bo)

    # permute wo rows to match gather order: lhsT chunk jr=(4j+r) holds head 4r+j
    perm = np.concatenate([
        np.arange(HD * (4 * r + j), HD * (4 * r + j) + HD)
        for j in range(GH) for r in range(4)
    ])
    wo_p = wo[perm, :]

    in_maps = []
    for c in range(8):
        b, g = divmod(c, 4)
        sl = slice(DG * g, DG * (g + 1))
        in_maps.append({
            "xT": np.ascontiguousarray(x[b].T).astype(np.float16),
            "wq": wq[:, sl].astype(np.float16),
            "bq": (bq[sl] * INV_SQRT_HD).astype(np.float16),
            "wk": wk[:, sl].astype(np.float16),
            "bk": bk[sl].astype(np.float16),
            "wv": wv[:, sl].astype(np.float16),
            "bv": bv[sl].astype(np.float16),
            "ckT": np.ascontiguousarray(cache_k[b][:, sl].T).astype(np.float16),
            "cv": cache_v[b][:, sl].astype(np.float16),
            "wo": wo_p[:, sl].astype(np.float16),
            "bo": bo[sl].astype(np.float16),
        })
    return in_maps


def assemble(results):
    out = np.empty((B, S, D), np.float32)
    for c in range(8):
        b, g = divmod(c, 4)
        out[b, :, DG * g:DG * (g + 1)] = results[c]["y"]
    return out


def kernel(**inputs):
    nc = get_built()
    in_maps = make_in_maps(**inputs)
    res = run_bass_kernel_spmd(nc, in_maps, core_ids=list(range(8)))
    return assemble(res.results)
